# revision 4
# baseline (speedup 1.0000x reference)
"""Trainium2 Bass kernel for nn_Attention_85005992722686.

Head-sharded tensor-parallel causal attention over 8 NeuronCores.
Core c owns heads {2c, 2c+1}; layernorms are algebraically folded:

  y = softmax(causal((LN(x;g,b) @ Wq) (LN(x;gc,bc) @ Wk)^T / 8)) @ (LN(x) @ Wv) @ Wo

v2 design (cost-model driven):
  - x streamed in bf16, blocked [D, 16*130] (128 data cols + 2 ones cols per
    block) -> gram stats matmuls are [128,130] bf16 (half the f32r cost).
  - projections in bf16 (W bf16); rank-1 mean corrections stay f32r.
  - q,k kept f32r for logit precision; v/p/attn/Wo/y in bf16.
  - scores for both heads land in one 2-bank PSUM tile -> single Exp per
    j-tile covering both heads (halves Act fixed overheads).
  - PV computed in natural layout (stationary = p block, moving = v_aug 66
    cols): ~half the PE rows of the transposed form, and the softmax
    denominator lands per-partition -> normalize is a cheap per-partition
    tensor_scalar fused into eviction, then one [128,128] PE transpose per
    i-tile back to attn^T for the output projection.
  - y written bf16 (host accumulates partials in f32).
  - per-chunk software pipeline: gram/stats/proj/attention interleaved per
    512-token chunk; x and qkv buffers double-buffered across batch.
"""
import sys
sys.path.insert(0, '/opt/trn_rl_repo')
import numpy as np
import ml_dtypes
import concourse.bass as bass
import concourse.bacc as bacc
import concourse.tile as tile
from concourse import mybir
from concourse.bass_utils import run_bass_kernel_spmd

F32 = mybir.dt.float32
F32R = mybir.dt.float32r
BF16 = mybir.dt.bfloat16
AF = mybir.ActivationFunctionType
ALU = mybir.AluOpType

B, N, D = 2, 2048, 1024
H, DH = 16, 64
EPS = 1e-5
NCORES = 8
HD = 128          # head-dim slice per core (2 heads x 64)
KT = D // 128     # 8 k-tiles over model dim
NT = N // 128     # 16 n-tiles
NCH = N // 512    # 4 n-chunks of 512
BLK = 130         # x block: 128 data cols + 2 ones cols
TPC = 4 * BLK     # 520 cols per (kt, chunk) x-tile

TRACE = False
TRACE_KWARGS = {}
LAST_RESULTS = None


def _build_program(with_bias):
    nc = bacc.Bacc("TRN2", target_bir_lowering=False, debug=False,
                   num_devices=NCORES)
    # ---------------- dram io ----------------
    xt_d = nc.dram_tensor("xt", [B, D, NT * BLK], BF16, kind="ExternalInput")
    wqkv_d = nc.dram_tensor("wqkv", [D, 3 * HD], BF16, kind="ExternalInput")
    wo_d = nc.dram_tensor("wo", [HD, D], BF16, kind="ExternalInput")
    # aux row: [ncs_q | ncs_k | ncs_v | ones] each 128 wide
    aux_d = nc.dram_tensor("aux", [1, 512], F32R, kind="ExternalInput")
    tri_d = nc.dram_tensor("tri", [128, 128], BF16, kind="ExternalInput")
    identf_d = nc.dram_tensor("identf", [128, 128], F32, kind="ExternalInput")
    identb_d = nc.dram_tensor("identb", [128, 128], BF16, kind="ExternalInput")
    if with_bias:
        bias_d = nc.dram_tensor("biasr", [1, 384], F32R, kind="ExternalInput")
    y_d = nc.dram_tensor("y", [B, N, D], BF16, kind="ExternalOutput")

    with tile.TileContext(nc) as tc:
        with tc.tile_pool(name="wpool", bufs=1) as wpool, \
             tc.tile_pool(name="xpool", bufs=2) as xpool, \
             tc.tile_pool(name="big", bufs=2) as bigp, \
             tc.tile_pool(name="small", bufs=2) as smallp, \
             tc.tile_pool(name="pstrip", bufs=3) as ppool, \
             tc.tile_pool(name="ypool", bufs=3) as ypool, \
             tc.tile_pool(name="psbig", bufs=2, space="PSUM") as psbig, \
             tc.tile_pool(name="pssm", bufs=2, space="PSUM") as pssm, \
             tc.tile_pool(name="pspv", bufs=1, space="PSUM") as pspv:

            # ---- b0 x tiles first (gate the first grams) ----
            xt_sb = {}
            for kt in range(KT):
                for c4 in range(NCH):
                    t = xpool.tile([128, TPC], BF16, name=f"xt0_{kt}_{c4}",
                                   tag=f"xt{kt}_{c4}")
                    nc.sync.dma_start(
                        t[:], xt_d.ap()[0, kt * 128:(kt + 1) * 128,
                                        c4 * TPC:(c4 + 1) * TPC])
                    xt_sb[0, kt, c4] = t

            # ---- statics ----
            w_sb = {}
            for kt in range(KT):
                t = wpool.tile([128, 3 * HD], BF16, name=f"wqkv{kt}")
                nc.sync.dma_start(t[:], wqkv_d.ap()[kt * 128:(kt + 1) * 128, :])
                for ti, nm in enumerate(("q", "k", "v")):
                    w_sb[nm, kt] = t[:, ti * HD:(ti + 1) * HD]
            identf_sb = wpool.tile([128, 128], F32, name="identf_sb")
            nc.sync.dma_start(identf_sb[:], identf_d.ap()[:, :])
            identb_sb = wpool.tile([128, 128], BF16, name="identb_sb")
            nc.sync.dma_start(identb_sb[:], identb_d.ap()[:, :])
            aux_sb = wpool.tile([1, 512], F32R, name="aux_sb")
            nc.sync.dma_start(aux_sb[:], aux_d.ap()[:, :])
            tri_sb = wpool.tile([128, 128], BF16, name="tri_sb")
            nc.sync.dma_start(tri_sb[:], tri_d.ap()[:, :])
            wo_sb = wpool.tile([HD, D], BF16, name="wo_sb")
            nc.sync.dma_start(wo_sb[:], wo_d.ap()[:, :])
            if with_bias:
                bias_sb = wpool.tile([1, 384], F32R, name="bias_sb")
                nc.sync.dma_start(bias_sb[:], bias_d.ap()[:, :])
            ones_row = aux_sb[0:1, 384:512]        # [1, 128] of ones (f32r)

            for b in range(B):
                if b > 0:
                    for kt in range(KT):
                        for c4 in range(NCH):
                            t = xpool.tile([128, TPC], BF16,
                                           name=f"xt{b}_{kt}_{c4}",
                                           tag=f"xt{kt}_{c4}")
                            nc.sync.dma_start(
                                t[:], xt_d.ap()[b, kt * 128:(kt + 1) * 128,
                                                c4 * TPC:(c4 + 1) * TPC])
                            xt_sb[b, kt, c4] = t

                def xblock(kt, p):
                    """[128, 130] block p (incl ones cols) of k-tile kt"""
                    c4, i4 = p // 4, p % 4
                    return xt_sb[b, kt, c4][:, i4 * BLK:(i4 + 1) * BLK]

                def xchunk(kt, c4):
                    """512 data cols of chunk c4 as 4x128 blocked AP"""
                    v = xt_sb[b, kt, c4].rearrange("p (a c) -> p a c", c=BLK)
                    return v[:, :, 0:128]

                # ---- per-batch tiles ----
                qT = bigp.tile([HD, N], F32R, name=f"qT{b}", tag="qT")
                kTt = bigp.tile([HD, N], F32R, name=f"kT{b}", tag="kT")
                vT = bigp.tile([HD, N], BF16, name=f"vT{b}", tag="vT")
                vnat = bigp.tile([128, NT * 132], BF16, name=f"vnat{b}",
                                 tag="vnat")
                ahT = bigp.tile([HD, N], BF16, name=f"ahT{b}", tag="ahT")
                mean_st = smallp.tile([128, 48], F32, name=f"mst{b}", tag="mst")
                mean_row = smallp.tile([1, N], F32R, name=f"mrow{b}", tag="mrow")
                s_row = smallp.tile([1, N], F32R, name=f"srow{b}", tag="srow")
                if with_bias:
                    std_row = smallp.tile([1, N], F32R, name=f"drow{b}",
                                          tag="drow")

                # v_aug ones cols (cols 64:66 of each 66-block)
                vv = vnat.rearrange("p (n u c) -> p n u c", u=2, c=66)
                tri16 = tri_sb[:, 0:32].rearrange("p (a c) -> p a c", c=2)
                for u in range(2):
                    nc.scalar.activation(vv[:, :, u, 64:66], tri16, AF.Copy,
                                         bias=1.0, scale=0.0)

                def v_aug(jt, h):
                    return vnat[:, jt * 132 + h * 66: jt * 132 + h * 66 + 66]

                for c4 in range(NCH):
                    sl = slice(c4 * 512, (c4 + 1) * 512)
                    cm = mean_st[:, 12 * c4:12 * c4 + 4]
                    cr = mean_st[:, 12 * c4 + 4:12 * c4 + 8]
                    cd = mean_st[:, 12 * c4 + 8:12 * c4 + 12]
                    # -- gram matmuls (PE, bf16 [128,130]) --
                    g_tiles = []
                    for i4 in range(4):
                        p = 4 * c4 + i4
                        g_ps = pssm.tile([128, BLK], F32, name=f"g{b}_{c4}_{i4}",
                                         tag="sm")
                        for kt in range(KT):
                            nc.tensor.matmul(
                                g_ps[:], xblock(kt, p)[:, 0:128], xblock(kt, p),
                                start=(kt == 0), stop=(kt == KT - 1))
                        g_tiles.append((g_ps, i4))
                    # -- projection main matmuls (bf16) --
                    prQK = psbig.tile([128, 1024], F32, name=f"prqk{b}{c4}",
                                      tag="big")
                    for kt in range(KT):
                        nc.tensor.matmul(prQK[:, 0:512], w_sb["q", kt],
                                         xchunk(kt, c4),
                                         start=(kt == 0), stop=False)
                    for kt in range(KT):
                        nc.tensor.matmul(prQK[:, 512:1024], w_sb["k", kt],
                                         xchunk(kt, c4),
                                         start=(kt == 0), stop=False)
                    prV = pssm.tile([128, 512], F32, name=f"prv{b}{c4}",
                                    tag="sm")
                    for kt in range(KT):
                        nc.tensor.matmul(prV[:], w_sb["v", kt], xchunk(kt, c4),
                                         start=(kt == 0), stop=False)
                    # -- stats extraction (DVE, overlaps proj matmuls) --
                    scratch = smallp.tile([128, 128], F32, name=f"scr{b}{c4}",
                                          tag="scr")
                    for g_ps, i4 in g_tiles:
                        nc.vector.scalar_tensor_tensor(
                            out=scratch[:, 0:128],
                            in0=g_ps[:, 0:128],
                            scalar=1.0 / D,
                            in1=identf_sb[:],
                            op0=ALU.mult, op1=ALU.mult,
                            accum_out=cd[:, i4:i4 + 1])
                        nc.vector.tensor_scalar(
                            out=cm[:, i4:i4 + 1],
                            in0=g_ps[:, 128:129], scalar1=1.0 / D, scalar2=None,
                            op0=ALU.mult)
                    # -- stats math --
                    sq = smallp.tile([128, 4], F32, name=f"sq{b}_{c4}", tag="sq")
                    nc.vector.tensor_mul(sq[:], cm, cm)
                    nc.vector.scalar_tensor_tensor(
                        out=cd, in0=cd, scalar=EPS, in1=sq[:],
                        op0=ALU.add, op1=ALU.subtract)
                    nc.scalar.activation(cd, cd, AF.Sqrt)
                    nc.vector.reciprocal(cr, cd)
                    # -- transpose stats block to rows (PE) --
                    st_ps = pssm.tile([12, 128], F32, name=f"stp{b}_{c4}",
                                      tag="sm")
                    nc.tensor.transpose(st_ps[:],
                                        mean_st[:, 12 * c4:12 * c4 + 12],
                                        identf_sb[:])
                    st_T = smallp.tile([12, 128], F32R, name=f"stT{b}_{c4}",
                                       tag="stT")
                    nc.vector.tensor_copy(st_T[:], st_ps[:])
                    nc.sync.dma_start(mean_row[0:1, sl], st_T[0:4, :])
                    nc.sync.dma_start(s_row[0:1, sl], st_T[4:8, :])
                    if with_bias:
                        nc.sync.dma_start(std_row[0:1, sl], st_T[8:12, :])
                    # -- rstd broadcast (PE + DVE evict) --
                    bc_ps = pssm.tile([128, 512], F32, name=f"bc{b}_{c4}",
                                      tag="sm")
                    nc.tensor.matmul(bc_ps[:], ones_row, s_row[0:1, sl],
                                     start=True, stop=True)
                    s_bc = smallp.tile([128, 512], F32, name=f"sbc{b}{c4}",
                                       tag="sbc")
                    nc.vector.tensor_copy(s_bc[:], bc_ps[:])
                    # -- rank-1 corrections (f32r) + evictions --
                    for ti, nm in enumerate(("q", "k", "v")):
                        tgt = (prQK[:, 0:512], prQK[:, 512:1024], prV[:])[ti]
                        nc.tensor.matmul(
                            tgt, aux_sb[0:1, ti * 128:(ti + 1) * 128],
                            mean_row[0:1, sl],
                            start=False, stop=not with_bias)
                        if with_bias:
                            nc.tensor.matmul(
                                tgt, bias_sb[0:1, ti * 128:(ti + 1) * 128],
                                std_row[0:1, sl],
                                start=False, stop=True)
                    nc.vector.tensor_mul(qT[:, sl], prQK[:, 0:512], s_bc[:])
                    nc.vector.tensor_mul(kTt[:, sl], prQK[:, 512:1024], s_bc[:])
                    nc.vector.tensor_mul(vT[:, sl], prV[:], s_bc[:])
                    # -- v -> natural layout (PE transpose, bf16) --
                    vt_ps = pssm.tile([128, 512], BF16, name=f"vt{b}_{c4}",
                                      tag="sm")
                    for j in range(4):
                        nt = 4 * c4 + j
                        nc.tensor.transpose(
                            vt_ps[:, j * 128:(j + 1) * 128],
                            vT[:, nt * 128:(nt + 1) * 128], identb_sb[:])
                    src = vt_ps.rearrange("p (n u c) -> p n u c", u=2, c=64)
                    dst = vv[:, 4 * c4:4 * c4 + 4, :, 0:64]
                    nc.vector.tensor_copy(dst, src)

                    # ---- out projection for previous chunk (pipeline lag) ----
                    def out_proj(oc):
                        for it in range(4 * oc, 4 * oc + 4):
                            y_ps = psbig.tile([128, 1024], F32,
                                              name=f"yp{b}_{it}", tag="big")
                            for e in range(2):
                                nc.tensor.matmul(
                                    y_ps[:, e * 512:(e + 1) * 512],
                                    ahT[:, it * 128:(it + 1) * 128],
                                    wo_sb[:, e * 512:(e + 1) * 512],
                                    start=True, stop=True)
                            y_sb = ypool.tile([128, D], BF16, name=f"y{b}_{it}",
                                              tag="ysb")
                            nc.scalar.copy(y_sb[:, 0:512], y_ps[:, 0:512])
                            nc.vector.tensor_copy(y_sb[:, 512:1024],
                                                  y_ps[:, 512:1024])
                            nc.sync.dma_start(
                                y_d.ap()[b, it * 128:(it + 1) * 128, :], y_sb[:])

                    if c4 > 0:
                        out_proj(c4 - 1)

                    # ---------------- attention for chunk c4 ----------------
                    # NB: matmul start=True marks the whole 2KB PSUM zero
                    # region pending-zero, which would wipe sibling
                    # accumulations sharing the bank. So: exactly one
                    # start=True (first write) and one stop=True (last
                    # write) per pv bank per chunk; first write to each
                    # byte range overwrites via the pending-zero mechanism.
                    pv_t = {}
                    pv_t[0] = pspv.tile([128, 264], F32, name=f"pvA{b}{c4}",
                                        tag="pvA")
                    pv_t[1] = pspv.tile([128, 264], F32, name=f"pvB{b}{c4}",
                                        tag="pvB")
                    pv_started = [False, False]

                    def pv_slot(isub):
                        t = pv_t[isub // 2]
                        po = (isub % 2) * 132
                        return t[:, po:po + 132]

                    njt = 4 * c4 + 4
                    for jt in range(njt):
                        off = 0 if jt < 4 * c4 else (jt - 4 * c4) * 128
                        w = 512 - off
                        sc = psbig.tile([128, 1024], F32, name=f"sc{b}{c4}{jt}",
                                        tag="big")
                        for h in range(2):
                            nc.tensor.matmul(
                                sc[:, h * 512 + off:(h + 1) * 512],
                                kTt[h * 64:(h + 1) * 64,
                                    jt * 128:(jt + 1) * 128],
                                qT[h * 64:(h + 1) * 64,
                                   c4 * 512 + off:(c4 + 1) * 512],
                                start=True, stop=True)
                        p_sb = ppool.tile([128, 1024], BF16,
                                          name=f"p{b}{c4}{jt}", tag="p")
                        sc_v = sc.rearrange("p (u c) -> p u c", c=512)
                        p_v = p_sb.rearrange("p (u c) -> p u c", c=512)
                        nc.scalar.activation(p_v[:, :, off:512],
                                             sc_v[:, :, off:512], AF.Exp)
                        if jt >= 4 * c4:
                            # diagonal block: mask (keep j <= i)
                            for h in range(2):
                                blk = p_sb[:, h * 512 + off:h * 512 + off + 128]
                                nc.gpsimd.tensor_mul(blk, blk, tri_sb[:])
                        lo_isub = 0 if jt < 4 * c4 else jt - 4 * c4
                        for isub in range(lo_isub, 4):
                            for h in range(2):
                                bank = isub // 2
                                last_isub = 1 if bank == 0 else 3
                                nc.tensor.matmul(
                                    pv_slot(isub)[:, h * 66:h * 66 + 66],
                                    p_sb[:, h * 512 + isub * 128:
                                         h * 512 + (isub + 1) * 128],
                                    v_aug(jt, h),
                                    start=not pv_started[bank],
                                    stop=(isub == last_isub
                                          and jt == 4 * c4 + isub and h == 1))
                                pv_started[bank] = True
                        # normalize + transpose finished i-subtiles
                        if jt >= 4 * c4:
                            isub = jt - 4 * c4
                            it = 4 * c4 + isub
                            slot = pv_slot(isub)
                            den = slot.rearrange("p (u c) -> p u c",
                                                 c=66)[:, :, 64:65]
                            rd = smallp.tile([128, 2], F32, name=f"rd{b}{it}",
                                             tag="rd")
                            rdv = rd.rearrange("p (u c) -> p u c", c=1)
                            with nc.allow_low_precision(reason="softmax denom"):
                                nc.vector.reciprocal(rdv, den)
                            pv_sb = smallp.tile([128, 128], BF16,
                                                name=f"pvs{b}{it}", tag="pvs")
                            for h in range(2):
                                nc.vector.tensor_scalar(
                                    out=pv_sb[:, h * 64:(h + 1) * 64],
                                    in0=slot[:, h * 66:h * 66 + 64],
                                    scalar1=rd[:, h:h + 1], scalar2=None,
                                    op0=ALU.mult)
                            at_ps = pssm.tile([128, 128], BF16,
                                              name=f"at{b}{it}", tag="sm")
                            nc.tensor.transpose(at_ps[:], pv_sb[:],
                                                identb_sb[:])
                            nc.vector.tensor_copy(
                                ahT[:, it * 128:(it + 1) * 128], at_ps[:])

                    if c4 == NCH - 1:
                        out_proj(c4)

    nc.compile()
    return nc


_PROG_CACHE = {}


def _get_program(with_bias):
    key = with_bias
    if key not in _PROG_CACHE:
        _PROG_CACHE[key] = _build_program(with_bias)
    return _PROG_CACHE[key]


def kernel(x, ln_g, ln_b, lnc_g, lnc_b, Wq, Wkv, Wo):
    global LAST_RESULTS
    x = np.ascontiguousarray(np.asarray(x, dtype=np.float32))
    ln_g = np.asarray(ln_g, np.float32); ln_b = np.asarray(ln_b, np.float32)
    lnc_g = np.asarray(lnc_g, np.float32); lnc_b = np.asarray(lnc_b, np.float32)
    Wq = np.asarray(Wq, np.float32); Wkv = np.asarray(Wkv, np.float32)
    Wo = np.asarray(Wo, np.float32)
    scale = DH ** -0.5

    with_bias = bool(np.any(ln_b) or np.any(lnc_b))
    nc = _get_program(with_bias)

    # xT packed bf16 with ones cols: [B, D, 16*130]
    xt = np.empty((B, D, NT, BLK), ml_dtypes.bfloat16)
    xTt = np.transpose(x, (0, 2, 1))                     # [B, D, N]
    xt[:, :, :, 0:128] = xTt.reshape(B, D, NT, 128).astype(ml_dtypes.bfloat16)
    xt[:, :, :, 128:130] = 1.0
    xt = xt.reshape(B, D, NT * BLK)

    tri = np.triu(np.ones((128, 128), np.float32)).astype(ml_dtypes.bfloat16)
    identf = np.eye(128, dtype=np.float32)
    identb = np.eye(128, dtype=np.float32).astype(ml_dtypes.bfloat16)

    in_maps = []
    for c in range(NCORES):
        cs = slice(c * HD, (c + 1) * HD)
        Wq_eff = (ln_g[:, None] * Wq[:, cs] * scale).astype(ml_dtypes.bfloat16)
        Wk_eff = (lnc_g[:, None] * Wkv[:, :H * DH][:, cs]).astype(
            ml_dtypes.bfloat16)
        Wv_eff = (lnc_g[:, None] * Wkv[:, H * DH:][:, cs]).astype(
            ml_dtypes.bfloat16)
        aux = np.zeros((1, 512), np.float32)
        aux[0, 0:128] = -Wq_eff.astype(np.float32).sum(0)
        aux[0, 128:256] = -Wk_eff.astype(np.float32).sum(0)
        aux[0, 256:384] = -Wv_eff.astype(np.float32).sum(0)
        aux[0, 384:512] = 1.0
        m = {
            "xt": xt,
            "wqkv": np.ascontiguousarray(
                np.concatenate([Wq_eff, Wk_eff, Wv_eff], axis=1)),
            "wo": np.ascontiguousarray(Wo[cs, :]).astype(ml_dtypes.bfloat16),
            "aux": aux, "tri": tri, "identf": identf, "identb": identb,
        }
        if with_bias:
            br = np.zeros((1, 384), np.float32)
            br[0, 0:128] = ln_b @ Wq[:, cs] * scale
            br[0, 128:256] = lnc_b @ Wkv[:, :H * DH][:, cs]
            br[0, 256:384] = lnc_b @ Wkv[:, H * DH:][:, cs]
            m["biasr"] = br
        in_maps.append(m)

    res = run_bass_kernel_spmd(nc, in_maps, core_ids=list(range(NCORES)),
                               trace=TRACE, **TRACE_KWARGS)
    LAST_RESULTS = res
    y = res.results[0]["y"].astype(np.float32)
    for c in range(1, NCORES):
        y += res.results[c]["y"].astype(np.float32)
    return y
